# revision 18
# baseline (speedup 1.0000x reference)
"""Trainium2 Bass kernel for the DiffPool-style GCN forward pass.

Computation (dead softmax/pool branches of the reference are skipped):
    x1 = relu(Dh (A+I) Dh (x @ W1e) + b1e)
    x2 = relu(Dh (A+I) Dh (x1 @ W2e) + b2e)
    out = (graph_mean_pool(x2) @ Wlin) + blin          -> [64, 10] fp32

Aggregation is reassociated so raw features are aggregated first and the
dense W matmul runs once per dst window. The self-loop term is folded into
the PSUM accumulation as an identity-matmul seed (x*dinv for layer 1,
dinv*x1 for layer 2), so each window epilogue is a single DVE multiply.

Structure (v6 = v1's proven 2-phase layout + layer-1 speedups):
- Layer 1 runs on 64-wide dst windows with a host-prepared fp8
  edge-aligned stream (no gather); one-hot scatter matrices built on DVE.
- Layer 2 gathers dinv*x1 rows (bf16, SWDGE dma_gather, 1024 rows/call,
  4 queues) from tables built by two AllGathers (lo = local rows
  [0,2560), hi = rest), in TWO 128-wide window passes so the hi
  AllGather overlaps the lo pass. Partials accumulate via identity-seeded
  PSUM with ACT drains (ACT/PE never contend with the SWDGE Q7 port).
- Mean-pool partials are reduced with one AllReduce.
"""

import numpy as np
import ml_dtypes

N = 50000
E = 800000
G = 64
C = 128
C_OUT = 10
NCORES = 8
NLOC = N // NCORES          # 6250
WD1 = 64                    # layer-1 dst window width
W1 = (NLOC + WD1 - 1) // WD1   # 98
NPAIR = (W1 + 1) // 2       # 49 pairs (128 dst each)
WD2 = 128                   # layer-2 dst window width
W2 = (NLOC + WD2 - 1) // WD2   # 49
NPAD = NPAIR * 128          # 6272
LOCUT = 2560                # lo/hi split (20 l2-windows; AG-lo fires early)
HILEN = NLOC - LOCUT        # 3690
NTABLO = LOCUT * NCORES
NTABHI = HILEN * NCORES
MAX_CALL_CHUNKS = 8
XE_SLAB = 32
OH_GROUP = 32
NQ = 4

BF16 = ml_dtypes.bfloat16
FP8 = ml_dtypes.float8_e4m3fn

_CACHE = {}


def _build_program(plan):
    import concourse.bacc as bacc
    import concourse.mybir as mybir
    import concourse.tile as tile
    from concourse import library_config
    from concourse.bass_interp import get_hw_module
    from concourse.tile_rust import add_dep_helper

    f32 = mybir.dt.float32
    bf16 = mybir.dt.bfloat16
    fp8 = mybir.dt.float8e4
    i16 = mybir.dt.int16
    Relu = mybir.ActivationFunctionType.Relu
    Copy = mybir.ActivationFunctionType.Copy
    Mult = mybir.AluOpType.mult

    f_chunks = plan["f_chunks"]      # [W1] layer-1 chunks per window
    win_f_base = plan["win_f_base"]
    TCH = plan["TCH"]                # aligned to OH_GROUP
    a_chunks = plan["a_chunks"]      # [W2] layer-2 lo
    b_chunks = plan["b_chunks"]      # [W2] hi
    calls_lo = plan["calls_lo"]
    calls_hi = plan["calls_hi"]
    win_lo_base = plan["win_lo_base"]
    win_hi_base = plan["win_hi_base"]
    TL = plan["TL"]
    TC = plan["total_drel_cols"]
    TIC = plan["total_idxcols"]

    nc = bacc.Bacc("TRN2", target_bir_lowering=False, debug=False,
                   num_devices=NCORES, num_swdge_queues=NQ,
                   dynamic_dma_scratch_size=49152)

    # ---- I/O ----
    xe_in = nc.dram_tensor("xedge", [C, TCH * C], fp8, kind="ExternalInput")
    xtl1_in = nc.dram_tensor("xTl1", [C, NPAD], bf16, kind="ExternalInput")
    dvr_in = nc.dram_tensor("dinvrow", [C, NPAD], bf16, kind="ExternalInput")
    idx_in = nc.dram_tensor("idx16", [C, TIC], i16, kind="ExternalInput")
    drel_in = nc.dram_tensor("drelb", [C, TC], bf16, kind="ExternalInput")
    iota_in = nc.dram_tensor("iotab", [C, C], bf16, kind="ExternalInput")
    sel_in = nc.dram_tensor("selg", [C, NPAIR * G], bf16, kind="ExternalInput")
    ident_in = nc.dram_tensor("identb", [C, C], bf16, kind="ExternalInput")
    w1_in = nc.dram_tensor("w1e", [C, C], bf16, kind="ExternalInput")
    w2_in = nc.dram_tensor("w2e", [C, C], bf16, kind="ExternalInput")
    wlin_in = nc.dram_tensor("wlin", [C, C_OUT], bf16, kind="ExternalInput")
    b1_in = nc.dram_tensor("b1row", [1, C], bf16, kind="ExternalInput")
    b2_in = nc.dram_tensor("b2row", [1, C], bf16, kind="ExternalInput")
    ones_in = nc.dram_tensor("ones1", [1, C], bf16, kind="ExternalInput")
    dinvw_in = nc.dram_tensor("dinvw", [C, NPAIR], f32, kind="ExternalInput")
    blin_in = nc.dram_tensor("blinb", [G, C_OUT], f32, kind="ExternalInput")
    icnt_in = nc.dram_tensor("invcnt", [G, 1], f32, kind="ExternalInput")
    out_t = nc.dram_tensor("out", [G, C_OUT], f32, kind="ExternalOutput")

    with tile.TileContext(nc) as tc:
        with tc.tile_pool(name="res", bufs=1) as res, \
             tc.tile_pool(name="gp", bufs=6) as gp, \
             tc.tile_pool(name="xep", bufs=3) as xep, \
             tc.tile_pool(name="ohp", bufs=3) as ohp, \
             tc.tile_pool(name="st2", bufs=3) as st2p, \
             tc.tile_pool(name="hx", bufs=4) as hxp, \
             tc.tile_pool(name="psw", bufs=2, space="PSUM") as psw, \
             tc.tile_pool(name="psd", bufs=2, space="PSUM") as psd, \
             tc.tile_pool(name="pstr", bufs=1, space="PSUM") as pstr, \
             tc.tile_pool(name="psp", bufs=1, space="PSUM") as psp, \
             tc.tile_pool(name="dram", bufs=1, space="DRAM") as dram:

            lib = nc.gpsimd.load_library(library_config.mlp)

            def load_res(name, src, shape, dt=f32):
                t = res.tile(shape, dt, tag=name, name=name)
                nc.sync.dma_start(out=t[:], in_=src[:])
                return t

            drel = load_res("r_drel", drel_in, [C, TC], bf16)
            iota = load_res("r_iota", iota_in, [C, C], bf16)
            xTl1 = load_res("r_xtl1", xtl1_in, [C, NPAD], bf16)
            dinvrow = load_res("r_dvr", dvr_in, [C, NPAD], bf16)
            identb = load_res("r_id", ident_in, [C, C], bf16)
            w1 = load_res("r_w1", w1_in, [C, C], bf16)
            w2 = load_res("r_w2", w2_in, [C, C], bf16)
            wlin = load_res("r_wl", wlin_in, [C, C_OUT], bf16)
            bias1 = load_res("r_b1", b1_in, [1, C], bf16)
            bias2 = load_res("r_b2", b2_in, [1, C], bf16)
            ones1 = load_res("r_on", ones_in, [1, C], bf16)
            dinvw = load_res("r_dw", dinvw_in, [C, NPAIR])
            blinb = load_res("r_bl", blin_in, [G, C_OUT])
            icnt = load_res("r_ic", icnt_in, [G, 1])

            x1T1 = res.tile([C, NPAD], bf16)   # dinv * x1^T (layer-2 seed)
            accT = res.tile([C, NPAD], bf16)   # pass-A partials + self

            # ---- DRAM buffers ----
            ag2a_in = dram.tile([LOCUT, C], bf16)
            ag2b_in = dram.tile([HILEN, C], bf16)
            lo2 = dram.tile([NTABLO, C], bf16)
            hi2 = dram.tile([NTABHI, C], bf16)
            ar_in = dram.tile([C, G], f32)
            ar_out = dram.tile([C, G], f32)
            rg = [list(range(NCORES))]

            def allgather(src, dst):
                nc.gpsimd.collective_compute(
                    "AllGather", mybir.AluOpType.bypass, replica_groups=rg,
                    ins=[src.opt()], outs=[dst.opt()])

            # ---- one-hot builder: cols < TCH are 64 wide, rest 128 ----
            ohtiles = {}

            def ensure_oh(gidx):
                g0 = (gidx // OH_GROUP) * OH_GROUP
                oht = ohtiles.get(g0)
                wd = WD1 if g0 < TCH else WD2
                if oht is None:
                    take = min(OH_GROUP, TC - g0)
                    oht = ohp.tile([C, OH_GROUP * WD2], bf16, tag="oh",
                                   name="ohbuf")
                    dcols = drel[:, g0:g0 + take]
                    nc.vector.tensor_tensor(
                        out=oht[:, 0:take * wd]
                            .rearrange("p (k m) -> p k m", m=wd),
                        in0=dcols.unsqueeze(2).to_broadcast([C, take, wd]),
                        in1=iota[:, 0:wd].unsqueeze(1)
                            .to_broadcast([C, take, wd]),
                        op=mybir.AluOpType.is_equal)
                    ohtiles[g0] = oht
                    if len(ohtiles) > 2:
                        ohtiles.pop(next(iter(ohtiles)))
                return oht, (gidx - g0), wd

            # ---- layer-1 fp8 edge stream ----
            xetiles = {}

            def ensure_xe(s):
                g0 = (s // XE_SLAB) * XE_SLAB
                xt_ = xetiles.get(g0)
                if xt_ is None:
                    take = min(XE_SLAB, TCH - g0)
                    xt_ = xep.tile([C, XE_SLAB * C], fp8, tag="xe",
                                   name="xebuf")
                    nc.sync.dma_start(out=xt_[:, 0:take * C],
                                      in_=xe_in[:, g0 * C:(g0 + take) * C])
                    xetiles[g0] = xt_
                    if len(xetiles) > 2:
                        xetiles.pop(next(iter(xetiles)))
                return xt_, s - g0

            # ===== layer 1: 64-wide windows, pairs of 2, fused dense =====
            stage2 = {"t": None, "p0": 0, "ag2a": None}
            idx16 = None
            selg = None

            def flush2(pend):
                """Flush stage pairs [p0, pend) to ag2a/ag2b (split LOCUT)."""
                if stage2["t"] is None:
                    return
                p0 = stage2["p0"]
                r0 = p0 * 128
                r1 = min(pend * 128, NLOC)
                r = r0
                while r < r1:
                    if r < LOCUT:
                        tab, trow, end = ag2a_in, r, min(r1, LOCUT)
                    else:
                        tab, trow, end = ag2b_in, r - LOCUT, r1
                    nrow = end - r
                    k0 = (r - r0) // 128
                    nfull = nrow // 128
                    if nfull > 0:
                        nc.sync.dma_start(
                            out=tab[trow:trow + nfull * 128, :]
                                .rearrange("(k p) c -> p k c", p=128),
                            in_=stage2["t"][:, k0 * C:(k0 + nfull) * C]
                                .rearrange("p (k c) -> p k c", c=C))
                    if nfull * 128 < nrow:
                        rem = nrow - nfull * 128
                        nc.sync.dma_start(
                            out=tab[trow + nfull * 128:trow + nrow, :],
                            in_=stage2["t"][0:rem,
                                            (k0 + nfull) * C:
                                            (k0 + nfull + 1) * C])
                    r = end
                stage2["t"] = None

            for pair in range(NPAIR):
                zt = hxp.tile([C, C], bf16, tag="z")
                for wi in range(2):
                    w = 2 * pair + wi
                    cw = f_chunks[w]
                    cols = slice(w * WD1, (w + 1) * WD1)
                    psA = psw.tile([C, WD2], f32, space="PSUM", tag="pw")
                    nc.tensor.matmul(out=psA[:, 0:WD1], lhsT=identb[:],
                                     rhs=xTl1[:, cols],
                                     start=True, stop=(cw == 0))
                    for k in range(cw):
                        sidx = win_f_base[w] + k
                        xe_t, xoff = ensure_xe(sidx)
                        oht, ooff, wd = ensure_oh(sidx)
                        nc.tensor.matmul(
                            out=psA[:, 0:WD1],
                            lhsT=xe_t[:, xoff * C:(xoff + 1) * C],
                            rhs=oht[:, ooff * wd:(ooff + 1) * wd],
                            start=False, stop=(k == cw - 1))
                    zc = slice(wi * WD1, (wi + 1) * WD1)
                    nc.vector.tensor_tensor(out=zt[:, zc], in0=psA[:, 0:WD1],
                                            in1=dinvrow[:, cols], op=Mult)
                ps2 = psd.tile([C, C], f32, space="PSUM", tag="pd")
                nc.tensor.matmul(out=ps2[:], lhsT=ones1[:], rhs=bias1[:],
                                 start=True, stop=False)
                nc.tensor.matmul(out=ps2[:], lhsT=zt[:], rhs=w1[:],
                                 start=False, stop=True)
                # stage rows: dinv * x1 (also the layer-2 seed, transposed)
                if stage2["t"] is None:
                    stage2["t"] = st2p.tile([C, 4 * C], bf16, tag="st2",
                                            name="st2buf")
                    stage2["p0"] = pair
                j = pair - stage2["p0"]
                nc.scalar.activation(stage2["t"][:, j * C:(j + 1) * C],
                                     ps2[:], Relu,
                                     scale=dinvw[:, pair:pair + 1])
                pt = pstr.tile([C, C], bf16, space="PSUM", tag="tps")
                nc.tensor.transpose(out=pt[:],
                                    in_=stage2["t"][:, j * C:(j + 1) * C],
                                    identity=identb[:])
                nc.scalar.activation(x1T1[:, pair * C:(pair + 1) * C],
                                     pt[:], Copy)
                if j == 3 or pair == NPAIR - 1 or (pair + 1) * 128 == LOCUT:
                    flush2(pair + 1)
                    if (pair + 1) * 128 >= LOCUT and stage2["ag2a"] is None:
                        allgather(ag2a_in, lo2)
                        idx16 = load_res("r_idx", idx_in, [C, TIC], i16)
                        selg = load_res("r_sel", sel_in,
                                        [C, NPAIR * G], bf16)
                        stage2["ag2a"] = True
            allgather(ag2b_in, hi2)

            # ===== layer 2: gather-based, two 128-wide passes =====
            state = {"tiles": {}, "next": [0, 0], "ci": [0]}

            def ensure_chunk(half, s):
                calls = calls_lo if half == 0 else calls_hi
                while True:
                    for (h2, st), (gt, nch) in state["tiles"].items():
                        if h2 == half and st <= s < st + nch:
                            return gt, s - st
                    st, nch, col = calls[state["next"][half]]
                    state["next"][half] += 1
                    gt = gp.tile([C, MAX_CALL_CHUNKS * C], bf16, tag="g",
                                 name="gbuf")
                    src_ap = lo2[:] if half == 0 else hi2[:]
                    ni = nch * 128
                    ci = state["ci"][0]
                    state["ci"][0] += 1
                    gi = nc.gpsimd.dma_gather(
                        gt[:, 0:nch * C].rearrange("p (k d) -> p k d", d=C),
                        src_ap, idx16[:, col:col + nch * 8],
                        ni, ni, C, single_packet=True, queue_num=ci % NQ)
                    add_dep_helper(gi.ins, lib.ins, False, "needs mlp lib")
                    state["tiles"][(half, st)] = (gt, nch)
                    if len(state["tiles"]) > 8:
                        state["tiles"].pop(next(iter(state["tiles"])))

            ps_pool = psp.tile([C, G], f32, space="PSUM", tag="pool")

            def edge_pass(half, base, cnt, colbase, final):
                for w in range(W2):
                    cw = cnt[w]
                    cols = slice(w * WD2, (w + 1) * WD2)
                    seed = accT if final else x1T1
                    ps = psw.tile([C, WD2], f32, space="PSUM", tag="pw")
                    nc.tensor.matmul(out=ps[:], lhsT=identb[:],
                                     rhs=seed[:, cols],
                                     start=True, stop=(cw == 0))
                    for k in range(cw):
                        gt, off = ensure_chunk(half, base[w] + k)
                        oht, ooff, wd = ensure_oh(colbase + base[w] + k)
                        nc.tensor.matmul(
                            out=ps[:],
                            lhsT=gt[:, off * C:(off + 1) * C],
                            rhs=oht[:, ooff * wd:(ooff + 1) * wd],
                            start=False, stop=(k == cw - 1))
                    if not final:
                        nc.scalar.activation(accT[:, cols], ps[:], Copy)
                    else:
                        zb = hxp.tile([C, C], bf16, tag="z")
                        nc.vector.tensor_tensor(out=zb[:], in0=ps[:],
                                                in1=dinvrow[:, cols],
                                                op=Mult)
                        ps2 = psd.tile([C, C], f32, space="PSUM", tag="pd")
                        nc.tensor.matmul(out=ps2[:], lhsT=ones1[:],
                                         rhs=bias2[:], start=True, stop=False)
                        nc.tensor.matmul(out=ps2[:], lhsT=zb[:], rhs=w2[:],
                                         start=False, stop=True)
                        x2t = hxp.tile([C, C], bf16, tag="xt")
                        nc.scalar.activation(x2t[:], ps2[:], Relu)
                        nc.tensor.matmul(out=ps_pool[:], lhsT=x2t[:],
                                         rhs=selg[:, w * G:(w + 1) * G],
                                         start=(w == 0), stop=(w == W2 - 1))

            edge_pass(0, win_lo_base, a_chunks, TCH, final=False)
            edge_pass(1, win_hi_base, b_chunks, TCH + TL, final=True)

            # ===== pooled all-reduce + final linear =====
            poolT = res.tile([C, G], f32)
            nc.vector.tensor_copy(out=poolT[:], in_=ps_pool[:])
            nc.sync.dma_start(out=ar_in[:], in_=poolT[:])
            nc.gpsimd.collective_compute(
                "AllReduce", mybir.AluOpType.add, replica_groups=rg,
                ins=[ar_in.opt()], outs=[ar_out.opt()])
            poolS = res.tile([C, G], f32)
            nc.sync.dma_start(out=poolS[:], in_=ar_out[:])
            poolb = res.tile([C, G], bf16)
            nc.vector.tensor_copy(out=poolb[:], in_=poolS[:])
            ps_f = psd.tile([G, C_OUT], f32, space="PSUM", tag="pd")
            nc.tensor.matmul(out=ps_f[:], lhsT=poolb[:], rhs=wlin[:],
                             start=True, stop=True)
            fin = res.tile([G, C_OUT], f32)
            nc.vector.tensor_scalar_mul(fin[:], in0=ps_f[:], scalar1=icnt[:])
            nc.vector.tensor_add(out=fin[:], in0=fin[:], in1=blinb[:])
            nc.sync.dma_start(out=out_t[:], in_=fin[:])

    nc.compile()
    nc.m = get_hw_module(nc.m)
    return nc


def _preprocess(edge_index, batch):
    src = np.asarray(edge_index[0], dtype=np.int64)
    dst = np.asarray(edge_index[1], dtype=np.int64)
    batch = np.asarray(batch, dtype=np.int64)

    deg = np.bincount(dst, minlength=N).astype(np.float64) + 1.0
    dinv = (1.0 / np.sqrt(deg)).astype(np.float32)
    counts = np.bincount(batch, minlength=G).astype(np.float64)
    inv_cnt = (1.0 / np.maximum(counts, 1.0)).astype(np.float32)

    order = np.argsort(dst, kind="stable")
    src_s = src[order]
    dst_s = dst[order]
    core_lo = np.searchsorted(dst_s, np.arange(NCORES) * NLOC)
    core_hi = np.searchsorted(dst_s, (np.arange(NCORES) + 1) * NLOC)

    per_core = []
    f_cnt = np.zeros((NCORES, W1), np.int64)
    a_cnt = np.zeros((NCORES, W2), np.int64)
    b_cnt = np.zeros((NCORES, W2), np.int64)
    for c in range(NCORES):
        s = src_s[core_lo[c]:core_hi[c]]
        d = dst_s[core_lo[c]:core_hi[c]] - c * NLOC
        owner = s // NLOC
        pos = s - owner * NLOC
        is_lo = pos < LOCUT
        row = np.where(is_lo, owner * LOCUT + pos,
                       owner * HILEN + (pos - LOCUT))
        win1 = d // WD1
        win2 = d // WD2
        w1lo = np.searchsorted(win1, np.arange(W1))
        w1hi = np.searchsorted(win1, np.arange(W1) + 1)
        w2lo = np.searchsorted(win2, np.arange(W2))
        w2hi = np.searchsorted(win2, np.arange(W2) + 1)
        wins1 = []
        for w in range(W1):
            sl = slice(w1lo[w], w1hi[w])
            wins1.append((s[sl], d[sl] - w * WD1))
            f_cnt[c, w] = w1hi[w] - w1lo[w]
        wins2 = []
        for w in range(W2):
            sl = slice(w2lo[w], w2hi[w])
            dw = d[sl] - w * WD2
            il = is_lo[sl]
            rw = row[sl]
            wins2.append((rw[il], dw[il], rw[~il], dw[~il]))
            a_cnt[c, w] = int(il.sum())
            b_cnt[c, w] = (w2hi[w] - w2lo[w]) - a_cnt[c, w]
        per_core.append((wins1, wins2))

    f_chunks = [int(-(-f_cnt[:, w].max() // 128)) for w in range(W1)]
    win_f_base = np.concatenate([[0], np.cumsum(f_chunks)])[:W1].astype(int).tolist()
    TCH = int(sum(f_chunks))
    TCH = ((TCH + OH_GROUP - 1) // OH_GROUP) * OH_GROUP  # align for builder

    a_chunks = [int(-(-a_cnt[:, w].max() // 128)) for w in range(W2)]
    b_chunks = [int(-(-b_cnt[:, w].max() // 128)) for w in range(W2)]
    win_lo_base = np.concatenate([[0], np.cumsum(a_chunks)])[:W2].astype(int).tolist()
    win_hi_base = np.concatenate([[0], np.cumsum(b_chunks)])[:W2].astype(int).tolist()
    TL = int(sum(a_chunks))
    TH = int(sum(b_chunks))
    total_drel_cols = TCH + TL + TH

    calls_lo, calls_hi = [], []
    idx_col = 0
    for done_target, calls in ((TL, calls_lo), (TH, calls_hi)):
        done = 0
        while done < done_target:
            take = min(MAX_CALL_CHUNKS, done_target - done)
            calls.append((done, take, idx_col))
            idx_col += take * 8
            done += take
    total_idxcols = idx_col

    plan = {"f_chunks": f_chunks, "win_f_base": win_f_base, "TCH": TCH,
            "a_chunks": a_chunks, "b_chunks": b_chunks,
            "calls_lo": calls_lo, "calls_hi": calls_hi,
            "win_lo_base": win_lo_base, "win_hi_base": win_hi_base,
            "TL": TL, "total_drel_cols": total_drel_cols,
            "total_idxcols": total_idxcols}

    return dinv, inv_cnt, plan, per_core


def _host_arrays(plan, per_core, batch, xs):
    f_chunks = plan["f_chunks"]
    a_chunks = plan["a_chunks"]
    b_chunks = plan["b_chunks"]
    win_f_base = plan["win_f_base"]
    win_lo_base = plan["win_lo_base"]
    win_hi_base = plan["win_hi_base"]
    TCH = plan["TCH"]
    TL = plan["TL"]
    TC = plan["total_drel_cols"]
    TIC = plan["total_idxcols"]
    TH = TC - TCH - TL

    xe_arrs, idx_arrs, drel_arrs, sel_arrs = [], [], [], []
    xsb = xs.astype(FP8)
    for c in range(NCORES):
        wins1, wins2 = per_core[c]
        xe_t = np.zeros((128, TCH * C), FP8)
        drel_t = np.full((128, TC), -1.0, np.float32)
        lo_idx = np.zeros(TL * 128, np.int16)
        hi_idx = np.zeros(TH * 128, np.int16)
        for w in range(W1):
            sw, dw = wins1[w]
            o = win_f_base[w]
            nr = len(sw)
            nch = f_chunks[w]
            buf = np.zeros((nch * 128, C), FP8)
            buf[:nr] = xsb[sw]
            xe_t[:, o * C:(o + nch) * C] = \
                buf.reshape(nch, 128, C).transpose(1, 0, 2).reshape(128, nch * C)
            fl = np.full(nch * 128, -1.0, np.float32)
            fl[:nr] = dw.astype(np.float32)
            drel_t[:, o:o + nch] = fl.reshape(nch, 128).T
        for w in range(W2):
            rw_lo, dw_lo, rw_hi, dw_hi = wins2[w]
            o = win_lo_base[w]
            lo_idx[o * 128:o * 128 + len(rw_lo)] = rw_lo.astype(np.int16)
            fl = np.full(a_chunks[w] * 128, -1.0, np.float32)
            fl[:len(dw_lo)] = dw_lo.astype(np.float32)
            drel_t[:, TCH + o:TCH + o + a_chunks[w]] = \
                fl.reshape(a_chunks[w], 128).T
            o = win_hi_base[w]
            hi_idx[o * 128:o * 128 + len(rw_hi)] = rw_hi.astype(np.int16)
            fh = np.full(b_chunks[w] * 128, -1.0, np.float32)
            fh[:len(dw_hi)] = dw_hi.astype(np.float32)
            drel_t[:, TCH + TL + o:TCH + TL + o + b_chunks[w]] = \
                fh.reshape(b_chunks[w], 128).T
        idx_t = np.zeros((128, TIC), np.int16)
        for half, calls in ((0, plan["calls_lo"]), (1, plan["calls_hi"])):
            src_idx = lo_idx if half == 0 else hi_idx
            for s0, take, col in calls:
                seg = src_idx[s0 * 128:(s0 + take) * 128]
                wrap = seg.reshape(take * 8, 16).T
                idx_t[:, col:col + take * 8] = np.tile(wrap, (8, 1))
        xe_arrs.append(xe_t)
        idx_arrs.append(idx_t)
        drel_arrs.append(drel_t.astype(BF16))
        bc = np.full(NPAD, -1.0, np.float32)
        bc[:NLOC] = batch[c * NLOC:(c + 1) * NLOC].astype(np.float32)
        sel = (bc.reshape(NPAIR, 128).T[:, :, None]
               == np.arange(G, dtype=np.float32)[None, None, :]).astype(BF16)
        sel_arrs.append(np.ascontiguousarray(sel.reshape(128, NPAIR * G)))
    return xe_arrs, idx_arrs, drel_arrs, sel_arrs


def kernel(**inputs):
    from concourse import bass_utils

    x = np.asarray(inputs["x"], dtype=np.float32)
    batch = np.asarray(inputs["batch"], dtype=np.int64)
    dinv, inv_cnt, plan, per_core = _preprocess(
        np.asarray(inputs["edge_index"]), batch)

    key = (tuple(plan["f_chunks"]), tuple(plan["a_chunks"]),
           tuple(plan["b_chunks"]))
    if key not in _CACHE:
        _CACHE.clear()
        _CACHE[key] = _build_program(plan)
    nc = _CACHE[key]

    b1r = np.asarray(inputs["b1e"], np.float32).reshape(1, C).astype(BF16)
    b2r = np.asarray(inputs["b2e"], np.float32).reshape(1, C).astype(BF16)
    ones1 = np.ones((1, C), np.float32).astype(BF16)
    blinb = np.tile(np.asarray(inputs["blin"], np.float32), (G, 1))
    identb = np.eye(C, dtype=np.float32).astype(BF16)
    iotab = np.tile(np.arange(C, dtype=np.float32), (C, 1)).astype(BF16)

    xs = x * dinv[:, None]          # D^{-1/2} X
    xe_arrs, idx_arrs, drel_arrs, sel_arrs = _host_arrays(
        plan, per_core, batch, xs)

    in_maps = []
    for c in range(NCORES):
        lo = c * NLOC
        x1l = np.zeros((C, NPAD), np.float32)
        x1l[:, :NLOC] = (x[lo:lo + NLOC]
                         * dinv[lo:lo + NLOC][:, None]).T
        dv_flat = np.zeros(NPAD, np.float32)
        dv_flat[:NLOC] = dinv[lo:lo + NLOC]
        dwp = dv_flat.reshape(NPAIR, 128).T.copy()
        in_maps.append({
            "xedge": xe_arrs[c],
            "xTl1": x1l.astype(BF16),
            "dinvrow": np.tile(dv_flat, (C, 1)).astype(BF16),
            "idx16": idx_arrs[c], "drelb": drel_arrs[c],
            "selg": sel_arrs[c],
            "identb": identb, "iotab": iotab,
            "w1e": np.asarray(inputs["W1e"], np.float32).astype(BF16),
            "w2e": np.asarray(inputs["W2e"], np.float32).astype(BF16),
            "wlin": np.asarray(inputs["Wlin"], np.float32).astype(BF16),
            "b1row": b1r, "b2row": b2r, "ones1": ones1,
            "dinvw": dwp,
            "blinb": blinb, "invcnt": inv_cnt.reshape(G, 1),
        })

    trace = bool(inputs.get("_trace", False))
    last_err = None
    for _attempt in range(3):
        try:
            res = bass_utils.run_bass_kernel_spmd(nc, in_maps,
                                                  core_ids=list(range(NCORES)),
                                                  trace=trace)
            kernel._last = res
            return np.asarray(res.results[0]["out"], dtype=np.float32)
        except Exception as e:  # transient device-state failures: retry
            last_err = e
    raise last_err


# revision 20
# speedup vs baseline: 1.0968x; 1.0968x over previous
"""Trainium2 Bass kernel for the DiffPool-style GCN forward pass.

Computation (dead softmax/pool branches of the reference are skipped):
    x1 = relu(Dh (A+I) Dh (x @ W1e) + b1e)
    x2 = relu(Dh (A+I) Dh (x1 @ W2e) + b2e)
    out = (graph_mean_pool(x2) @ Wlin) + blin          -> [64, 10] fp32

Aggregation is reassociated so raw features are aggregated first and the
dense W matmul runs once per dst window. The self-loop term is folded into
the PSUM accumulation as an identity-matmul seed (x*dinv for layer 1,
dinv*x1 for layer 2), so each window epilogue is a single DVE multiply.

Structure (v6 = v1's proven 2-phase layout + layer-1 speedups):
- Layer 1 runs on 64-wide dst windows with a host-prepared fp8
  edge-aligned stream (no gather); one-hot scatter matrices built on DVE.
- Layer 2 gathers dinv*x1 rows (bf16, SWDGE dma_gather, 1024 rows/call,
  4 queues) from tables built by two AllGathers (lo = local rows
  [0,2560), hi = rest), in TWO 128-wide window passes so the hi
  AllGather overlaps the lo pass. Partials accumulate via identity-seeded
  PSUM with ACT drains (ACT/PE never contend with the SWDGE Q7 port).
- Mean-pool partials are reduced with one AllReduce.
"""

import numpy as np
import ml_dtypes

N = 50000
E = 800000
G = 64
C = 128
C_OUT = 10
NCORES = 8
NLOC = N // NCORES          # 6250
WD1 = 64                    # layer-1 dst window width
W1 = (NLOC + WD1 - 1) // WD1   # 98
NPAIR = (W1 + 1) // 2       # 49 pairs (128 dst each)
WD2 = 128                   # layer-2 dst window width
W2 = (NLOC + WD2 - 1) // WD2   # 49
NPAD = NPAIR * 128          # 6272
LOCUT = 2176                # lo/hi split; 8*HILEN=32592 just fits int16
HILEN = NLOC - LOCUT        # 3690
NTABLO = LOCUT * NCORES
NTABHI = HILEN * NCORES
MAX_CALL_CHUNKS = 8
XE_SLAB = 32
OH_GROUP = 32
NQ = 4

BF16 = ml_dtypes.bfloat16
FP8 = ml_dtypes.float8_e4m3fn

_CACHE = {}


def _build_program(plan):
    import concourse.bacc as bacc
    import concourse.mybir as mybir
    import concourse.tile as tile
    from concourse import library_config
    from concourse.bass_interp import get_hw_module
    from concourse.tile_rust import add_dep_helper

    f32 = mybir.dt.float32
    bf16 = mybir.dt.bfloat16
    fp8 = mybir.dt.float8e4
    i16 = mybir.dt.int16
    Relu = mybir.ActivationFunctionType.Relu
    Copy = mybir.ActivationFunctionType.Copy
    Mult = mybir.AluOpType.mult

    f_chunks = plan["f_chunks"]      # [W1] layer-1 chunks per window
    win_f_base = plan["win_f_base"]
    TCH = plan["TCH"]                # aligned to OH_GROUP
    a_chunks = plan["a_chunks"]      # [W2] layer-2 lo
    b_chunks = plan["b_chunks"]      # [W2] hi
    calls_lo = plan["calls_lo"]
    calls_hi = plan["calls_hi"]
    win_lo_base = plan["win_lo_base"]
    win_hi_base = plan["win_hi_base"]
    TL = plan["TL"]
    TC = plan["total_drel_cols"]
    TIC = plan["total_idxcols"]

    nc = bacc.Bacc("TRN2", target_bir_lowering=False, debug=False,
                   num_devices=NCORES, num_swdge_queues=NQ)

    # ---- I/O ----
    xe_in = nc.dram_tensor("xedge", [C, TCH * C], fp8, kind="ExternalInput")
    xtl1_in = nc.dram_tensor("xTl1", [C, NPAD], bf16, kind="ExternalInput")
    dvr_in = nc.dram_tensor("dinvrow", [C, NPAD], bf16, kind="ExternalInput")
    idx_in = nc.dram_tensor("idx16", [C, TIC], i16, kind="ExternalInput")
    drel_in = nc.dram_tensor("drelb", [C, TC], bf16, kind="ExternalInput")
    iota_in = nc.dram_tensor("iotab", [C, C], bf16, kind="ExternalInput")
    sel_in = nc.dram_tensor("selg", [C, NPAIR * G], bf16, kind="ExternalInput")
    ident_in = nc.dram_tensor("identb", [C, C], bf16, kind="ExternalInput")
    w1_in = nc.dram_tensor("w1e", [C, C], bf16, kind="ExternalInput")
    w2_in = nc.dram_tensor("w2e", [C, C], bf16, kind="ExternalInput")
    wlin_in = nc.dram_tensor("wlin", [C, C_OUT], bf16, kind="ExternalInput")
    b1_in = nc.dram_tensor("b1row", [1, C], bf16, kind="ExternalInput")
    b2_in = nc.dram_tensor("b2row", [1, C], bf16, kind="ExternalInput")
    ones_in = nc.dram_tensor("ones1", [1, C], bf16, kind="ExternalInput")
    dinvw_in = nc.dram_tensor("dinvw", [C, NPAIR], f32, kind="ExternalInput")
    blin_in = nc.dram_tensor("blinb", [G, C_OUT], f32, kind="ExternalInput")
    icnt_in = nc.dram_tensor("invcnt", [G, 1], f32, kind="ExternalInput")
    out_t = nc.dram_tensor("out", [G, C_OUT], f32, kind="ExternalOutput")

    with tile.TileContext(nc) as tc:
        with tc.tile_pool(name="res", bufs=1) as res, \
             tc.tile_pool(name="gp", bufs=6) as gp, \
             tc.tile_pool(name="xep", bufs=3) as xep, \
             tc.tile_pool(name="ohp", bufs=3) as ohp, \
             tc.tile_pool(name="st2", bufs=3) as st2p, \
             tc.tile_pool(name="hx", bufs=4) as hxp, \
             tc.tile_pool(name="psw", bufs=2, space="PSUM") as psw, \
             tc.tile_pool(name="psd", bufs=2, space="PSUM") as psd, \
             tc.tile_pool(name="pstr", bufs=1, space="PSUM") as pstr, \
             tc.tile_pool(name="psp", bufs=1, space="PSUM") as psp, \
             tc.tile_pool(name="dram", bufs=1, space="DRAM") as dram:

            lib = nc.gpsimd.load_library(library_config.mlp)

            def load_res(name, src, shape, dt=f32):
                t = res.tile(shape, dt, tag=name, name=name)
                nc.sync.dma_start(out=t[:], in_=src[:])
                return t

            drel = load_res("r_drel", drel_in, [C, TC], bf16)
            iota = load_res("r_iota", iota_in, [C, C], bf16)
            xTl1 = load_res("r_xtl1", xtl1_in, [C, NPAD], bf16)
            dinvrow = load_res("r_dvr", dvr_in, [C, NPAD], bf16)
            identb = load_res("r_id", ident_in, [C, C], bf16)
            w1 = load_res("r_w1", w1_in, [C, C], bf16)
            w2 = load_res("r_w2", w2_in, [C, C], bf16)
            wlin = load_res("r_wl", wlin_in, [C, C_OUT], bf16)
            bias1 = load_res("r_b1", b1_in, [1, C], bf16)
            bias2 = load_res("r_b2", b2_in, [1, C], bf16)
            ones1 = load_res("r_on", ones_in, [1, C], bf16)
            dinvw = load_res("r_dw", dinvw_in, [C, NPAIR])
            blinb = load_res("r_bl", blin_in, [G, C_OUT])
            icnt = load_res("r_ic", icnt_in, [G, 1])

            x1T1 = res.tile([C, NPAD], bf16)   # dinv * x1^T (layer-2 seed)
            accT = res.tile([C, NPAD], bf16)   # pass-A partials + self

            # ---- DRAM buffers ----
            ag2a_in = dram.tile([LOCUT, C], bf16)
            ag2b_in = dram.tile([HILEN, C], bf16)
            lo2 = dram.tile([NTABLO, C], bf16, addr_space="Shared")
            hi2 = dram.tile([NTABHI, C], bf16, addr_space="Shared")
            ar_in = dram.tile([C, G], f32)
            ar_out = dram.tile([C, G], f32)
            rg = [list(range(NCORES))]

            def allgather(src, dst):
                nc.gpsimd.collective_compute(
                    "AllGather", mybir.AluOpType.bypass, replica_groups=rg,
                    ins=[src.opt()], outs=[dst.opt()])

            # ---- one-hot builder: cols < TCH are 64 wide, rest 128 ----
            ohtiles = {}

            def ensure_oh(gidx):
                g0 = (gidx // OH_GROUP) * OH_GROUP
                oht = ohtiles.get(g0)
                wd = WD1 if g0 < TCH else WD2
                if oht is None:
                    take = min(OH_GROUP, TC - g0)
                    oht = ohp.tile([C, OH_GROUP * WD2], bf16, tag="oh",
                                   name="ohbuf")
                    dcols = drel[:, g0:g0 + take]
                    nc.vector.tensor_tensor(
                        out=oht[:, 0:take * wd]
                            .rearrange("p (k m) -> p k m", m=wd),
                        in0=dcols.unsqueeze(2).to_broadcast([C, take, wd]),
                        in1=iota[:, 0:wd].unsqueeze(1)
                            .to_broadcast([C, take, wd]),
                        op=mybir.AluOpType.is_equal)
                    ohtiles[g0] = oht
                    if len(ohtiles) > 2:
                        ohtiles.pop(next(iter(ohtiles)))
                return oht, (gidx - g0), wd

            # ---- layer-1 fp8 edge stream ----
            xetiles = {}

            def ensure_xe(s):
                g0 = (s // XE_SLAB) * XE_SLAB
                xt_ = xetiles.get(g0)
                if xt_ is None:
                    take = min(XE_SLAB, TCH - g0)
                    xt_ = xep.tile([C, XE_SLAB * C], fp8, tag="xe",
                                   name="xebuf")
                    nc.sync.dma_start(out=xt_[:, 0:take * C],
                                      in_=xe_in[:, g0 * C:(g0 + take) * C])
                    xetiles[g0] = xt_
                    if len(xetiles) > 2:
                        xetiles.pop(next(iter(xetiles)))
                return xt_, s - g0

            # ===== layer 1: 64-wide windows, pairs of 2, fused dense =====
            stage2 = {"t": None, "p0": 0, "ag2a": None}
            idx16 = None
            selg = None

            def flush2(pend):
                """Flush stage pairs [p0, pend) to ag2a/ag2b (split LOCUT)."""
                if stage2["t"] is None:
                    return
                p0 = stage2["p0"]
                r0 = p0 * 128
                r1 = min(pend * 128, NLOC)
                r = r0
                while r < r1:
                    if r < LOCUT:
                        tab, trow, end = ag2a_in, r, min(r1, LOCUT)
                    else:
                        tab, trow, end = ag2b_in, r - LOCUT, r1
                    nrow = end - r
                    k0 = (r - r0) // 128
                    nfull = nrow // 128
                    if nfull > 0:
                        nc.sync.dma_start(
                            out=tab[trow:trow + nfull * 128, :]
                                .rearrange("(k p) c -> p k c", p=128),
                            in_=stage2["t"][:, k0 * C:(k0 + nfull) * C]
                                .rearrange("p (k c) -> p k c", c=C))
                    if nfull * 128 < nrow:
                        rem = nrow - nfull * 128
                        nc.sync.dma_start(
                            out=tab[trow + nfull * 128:trow + nrow, :],
                            in_=stage2["t"][0:rem,
                                            (k0 + nfull) * C:
                                            (k0 + nfull + 1) * C])
                    r = end
                stage2["t"] = None

            for pair in range(NPAIR):
                zt = hxp.tile([C, C], bf16, tag="z")
                for wi in range(2):
                    w = 2 * pair + wi
                    cw = f_chunks[w]
                    cols = slice(w * WD1, (w + 1) * WD1)
                    psA = psw.tile([C, WD2], f32, space="PSUM", tag="pw")
                    nc.tensor.matmul(out=psA[:, 0:WD1], lhsT=identb[:],
                                     rhs=xTl1[:, cols],
                                     start=True, stop=(cw == 0))
                    for k in range(cw):
                        sidx = win_f_base[w] + k
                        xe_t, xoff = ensure_xe(sidx)
                        oht, ooff, wd = ensure_oh(sidx)
                        nc.tensor.matmul(
                            out=psA[:, 0:WD1],
                            lhsT=xe_t[:, xoff * C:(xoff + 1) * C],
                            rhs=oht[:, ooff * wd:(ooff + 1) * wd],
                            start=False, stop=(k == cw - 1))
                    zc = slice(wi * WD1, (wi + 1) * WD1)
                    nc.vector.tensor_tensor(out=zt[:, zc], in0=psA[:, 0:WD1],
                                            in1=dinvrow[:, cols], op=Mult)
                ps2 = psd.tile([C, C], f32, space="PSUM", tag="pd")
                nc.tensor.matmul(out=ps2[:], lhsT=ones1[:], rhs=bias1[:],
                                 start=True, stop=False)
                nc.tensor.matmul(out=ps2[:], lhsT=zt[:], rhs=w1[:],
                                 start=False, stop=True)
                # stage rows: dinv * x1 (also the layer-2 seed, transposed)
                if stage2["t"] is None:
                    stage2["t"] = st2p.tile([C, 4 * C], bf16, tag="st2",
                                            name="st2buf")
                    stage2["p0"] = pair
                j = pair - stage2["p0"]
                nc.scalar.activation(stage2["t"][:, j * C:(j + 1) * C],
                                     ps2[:], Relu,
                                     scale=dinvw[:, pair:pair + 1])
                pt = pstr.tile([C, C], bf16, space="PSUM", tag="tps")
                nc.tensor.transpose(out=pt[:],
                                    in_=stage2["t"][:, j * C:(j + 1) * C],
                                    identity=identb[:])
                nc.scalar.activation(x1T1[:, pair * C:(pair + 1) * C],
                                     pt[:], Copy)
                if j == 3 or pair == NPAIR - 1 or (pair + 1) * 128 == LOCUT:
                    flush2(pair + 1)
                    if (pair + 1) * 128 >= LOCUT and stage2["ag2a"] is None:
                        allgather(ag2a_in, lo2)
                        idx16 = load_res("r_idx", idx_in, [C, TIC], i16)
                        selg = load_res("r_sel", sel_in,
                                        [C, NPAIR * G], bf16)
                        stage2["ag2a"] = True
            allgather(ag2b_in, hi2)

            # ===== layer 2: gather-based, two 128-wide passes =====
            state = {"tiles": {}, "next": [0, 0], "ci": [0]}

            def ensure_chunk(half, s):
                calls = calls_lo if half == 0 else calls_hi
                while True:
                    for (h2, st), (gt, nch) in state["tiles"].items():
                        if h2 == half and st <= s < st + nch:
                            return gt, s - st
                    st, nch, col = calls[state["next"][half]]
                    state["next"][half] += 1
                    gt = gp.tile([C, MAX_CALL_CHUNKS * C], bf16, tag="g",
                                 name="gbuf")
                    src_ap = lo2[:] if half == 0 else hi2[:]
                    ni = nch * 128
                    ci = state["ci"][0]
                    state["ci"][0] += 1
                    gi = nc.gpsimd.dma_gather(
                        gt[:, 0:nch * C].rearrange("p (k d) -> p k d", d=C),
                        src_ap, idx16[:, col:col + nch * 8],
                        ni, ni, C, single_packet=True, queue_num=ci % NQ)
                    add_dep_helper(gi.ins, lib.ins, False, "needs mlp lib")
                    state["tiles"][(half, st)] = (gt, nch)
                    if len(state["tiles"]) > 8:
                        state["tiles"].pop(next(iter(state["tiles"])))

            ps_pool = psp.tile([C, G], f32, space="PSUM", tag="pool")

            def edge_pass(half, base, cnt, colbase, final):
                for w in range(W2):
                    cw = cnt[w]
                    cols = slice(w * WD2, (w + 1) * WD2)
                    seed = accT if final else x1T1
                    ps = psw.tile([C, WD2], f32, space="PSUM", tag="pw")
                    nc.tensor.matmul(out=ps[:], lhsT=identb[:],
                                     rhs=seed[:, cols],
                                     start=True, stop=(cw == 0))
                    for k in range(cw):
                        gt, off = ensure_chunk(half, base[w] + k)
                        oht, ooff, wd = ensure_oh(colbase + base[w] + k)
                        nc.tensor.matmul(
                            out=ps[:],
                            lhsT=gt[:, off * C:(off + 1) * C],
                            rhs=oht[:, ooff * wd:(ooff + 1) * wd],
                            start=False, stop=(k == cw - 1))
                    if not final:
                        nc.scalar.activation(accT[:, cols], ps[:], Copy)
                    else:
                        zb = hxp.tile([C, C], bf16, tag="z")
                        nc.vector.tensor_tensor(out=zb[:], in0=ps[:],
                                                in1=dinvrow[:, cols],
                                                op=Mult)
                        ps2 = psd.tile([C, C], f32, space="PSUM", tag="pd")
                        nc.tensor.matmul(out=ps2[:], lhsT=ones1[:],
                                         rhs=bias2[:], start=True, stop=False)
                        nc.tensor.matmul(out=ps2[:], lhsT=zb[:], rhs=w2[:],
                                         start=False, stop=True)
                        x2t = hxp.tile([C, C], bf16, tag="xt")
                        nc.scalar.activation(x2t[:], ps2[:], Relu)
                        nc.tensor.matmul(out=ps_pool[:], lhsT=x2t[:],
                                         rhs=selg[:, w * G:(w + 1) * G],
                                         start=(w == 0), stop=(w == W2 - 1))

            edge_pass(0, win_lo_base, a_chunks, TCH, final=False)
            edge_pass(1, win_hi_base, b_chunks, TCH + TL, final=True)

            # ===== pooled all-reduce + final linear =====
            poolT = res.tile([C, G], f32)
            nc.vector.tensor_copy(out=poolT[:], in_=ps_pool[:])
            nc.sync.dma_start(out=ar_in[:], in_=poolT[:])
            nc.gpsimd.collective_compute(
                "AllReduce", mybir.AluOpType.add, replica_groups=rg,
                ins=[ar_in.opt()], outs=[ar_out.opt()])
            poolS = res.tile([C, G], f32)
            nc.sync.dma_start(out=poolS[:], in_=ar_out[:])
            poolb = res.tile([C, G], bf16)
            nc.vector.tensor_copy(out=poolb[:], in_=poolS[:])
            ps_f = psd.tile([G, C_OUT], f32, space="PSUM", tag="pd")
            nc.tensor.matmul(out=ps_f[:], lhsT=poolb[:], rhs=wlin[:],
                             start=True, stop=True)
            fin = res.tile([G, C_OUT], f32)
            nc.vector.tensor_scalar_mul(fin[:], in0=ps_f[:], scalar1=icnt[:])
            nc.vector.tensor_add(out=fin[:], in0=fin[:], in1=blinb[:])
            nc.sync.dma_start(out=out_t[:], in_=fin[:])

    nc.compile()
    nc.m = get_hw_module(nc.m)
    return nc


def _preprocess(edge_index, batch):
    src = np.asarray(edge_index[0], dtype=np.int64)
    dst = np.asarray(edge_index[1], dtype=np.int64)
    batch = np.asarray(batch, dtype=np.int64)

    deg = np.bincount(dst, minlength=N).astype(np.float64) + 1.0
    dinv = (1.0 / np.sqrt(deg)).astype(np.float32)
    counts = np.bincount(batch, minlength=G).astype(np.float64)
    inv_cnt = (1.0 / np.maximum(counts, 1.0)).astype(np.float32)

    order = np.argsort(dst, kind="stable")
    src_s = src[order]
    dst_s = dst[order]
    core_lo = np.searchsorted(dst_s, np.arange(NCORES) * NLOC)
    core_hi = np.searchsorted(dst_s, (np.arange(NCORES) + 1) * NLOC)

    per_core = []
    f_cnt = np.zeros((NCORES, W1), np.int64)
    a_cnt = np.zeros((NCORES, W2), np.int64)
    b_cnt = np.zeros((NCORES, W2), np.int64)
    for c in range(NCORES):
        s = src_s[core_lo[c]:core_hi[c]]
        d = dst_s[core_lo[c]:core_hi[c]] - c * NLOC
        owner = s // NLOC
        pos = s - owner * NLOC
        is_lo = pos < LOCUT
        row = np.where(is_lo, owner * LOCUT + pos,
                       owner * HILEN + (pos - LOCUT))
        win1 = d // WD1
        win2 = d // WD2
        w1lo = np.searchsorted(win1, np.arange(W1))
        w1hi = np.searchsorted(win1, np.arange(W1) + 1)
        w2lo = np.searchsorted(win2, np.arange(W2))
        w2hi = np.searchsorted(win2, np.arange(W2) + 1)
        wins1 = []
        for w in range(W1):
            sl = slice(w1lo[w], w1hi[w])
            wins1.append((s[sl], d[sl] - w * WD1))
            f_cnt[c, w] = w1hi[w] - w1lo[w]
        wins2 = []
        for w in range(W2):
            sl = slice(w2lo[w], w2hi[w])
            dw = d[sl] - w * WD2
            il = is_lo[sl]
            rw = row[sl]
            wins2.append((rw[il], dw[il], rw[~il], dw[~il]))
            a_cnt[c, w] = int(il.sum())
            b_cnt[c, w] = (w2hi[w] - w2lo[w]) - a_cnt[c, w]
        per_core.append((wins1, wins2))

    f_chunks = [int(-(-f_cnt[:, w].max() // 128)) for w in range(W1)]
    win_f_base = np.concatenate([[0], np.cumsum(f_chunks)])[:W1].astype(int).tolist()
    TCH = int(sum(f_chunks))
    TCH = ((TCH + OH_GROUP - 1) // OH_GROUP) * OH_GROUP  # align for builder

    a_chunks = [int(-(-a_cnt[:, w].max() // 128)) for w in range(W2)]
    b_chunks = [int(-(-b_cnt[:, w].max() // 128)) for w in range(W2)]
    win_lo_base = np.concatenate([[0], np.cumsum(a_chunks)])[:W2].astype(int).tolist()
    win_hi_base = np.concatenate([[0], np.cumsum(b_chunks)])[:W2].astype(int).tolist()
    TL = int(sum(a_chunks))
    TH = int(sum(b_chunks))
    total_drel_cols = TCH + TL + TH

    calls_lo, calls_hi = [], []
    idx_col = 0
    for done_target, calls in ((TL, calls_lo), (TH, calls_hi)):
        done = 0
        while done < done_target:
            take = min(MAX_CALL_CHUNKS, done_target - done)
            calls.append((done, take, idx_col))
            idx_col += take * 8
            done += take
    total_idxcols = idx_col

    plan = {"f_chunks": f_chunks, "win_f_base": win_f_base, "TCH": TCH,
            "a_chunks": a_chunks, "b_chunks": b_chunks,
            "calls_lo": calls_lo, "calls_hi": calls_hi,
            "win_lo_base": win_lo_base, "win_hi_base": win_hi_base,
            "TL": TL, "total_drel_cols": total_drel_cols,
            "total_idxcols": total_idxcols}

    return dinv, inv_cnt, plan, per_core


def _host_arrays(plan, per_core, batch, xs):
    f_chunks = plan["f_chunks"]
    a_chunks = plan["a_chunks"]
    b_chunks = plan["b_chunks"]
    win_f_base = plan["win_f_base"]
    win_lo_base = plan["win_lo_base"]
    win_hi_base = plan["win_hi_base"]
    TCH = plan["TCH"]
    TL = plan["TL"]
    TC = plan["total_drel_cols"]
    TIC = plan["total_idxcols"]
    TH = TC - TCH - TL

    xe_arrs, idx_arrs, drel_arrs, sel_arrs = [], [], [], []
    xsb = xs.astype(FP8)
    for c in range(NCORES):
        wins1, wins2 = per_core[c]
        xe_t = np.zeros((128, TCH * C), FP8)
        drel_t = np.full((128, TC), -1.0, np.float32)
        lo_idx = np.zeros(TL * 128, np.int16)
        hi_idx = np.zeros(TH * 128, np.int16)
        for w in range(W1):
            sw, dw = wins1[w]
            o = win_f_base[w]
            nr = len(sw)
            nch = f_chunks[w]
            buf = np.zeros((nch * 128, C), FP8)
            buf[:nr] = xsb[sw]
            xe_t[:, o * C:(o + nch) * C] = \
                buf.reshape(nch, 128, C).transpose(1, 0, 2).reshape(128, nch * C)
            fl = np.full(nch * 128, -1.0, np.float32)
            fl[:nr] = dw.astype(np.float32)
            drel_t[:, o:o + nch] = fl.reshape(nch, 128).T
        for w in range(W2):
            rw_lo, dw_lo, rw_hi, dw_hi = wins2[w]
            o = win_lo_base[w]
            lo_idx[o * 128:o * 128 + len(rw_lo)] = rw_lo.astype(np.int16)
            fl = np.full(a_chunks[w] * 128, -1.0, np.float32)
            fl[:len(dw_lo)] = dw_lo.astype(np.float32)
            drel_t[:, TCH + o:TCH + o + a_chunks[w]] = \
                fl.reshape(a_chunks[w], 128).T
            o = win_hi_base[w]
            hi_idx[o * 128:o * 128 + len(rw_hi)] = rw_hi.astype(np.int16)
            fh = np.full(b_chunks[w] * 128, -1.0, np.float32)
            fh[:len(dw_hi)] = dw_hi.astype(np.float32)
            drel_t[:, TCH + TL + o:TCH + TL + o + b_chunks[w]] = \
                fh.reshape(b_chunks[w], 128).T
        idx_t = np.zeros((128, TIC), np.int16)
        for half, calls in ((0, plan["calls_lo"]), (1, plan["calls_hi"])):
            src_idx = lo_idx if half == 0 else hi_idx
            for s0, take, col in calls:
                seg = src_idx[s0 * 128:(s0 + take) * 128]
                wrap = seg.reshape(take * 8, 16).T
                idx_t[:, col:col + take * 8] = np.tile(wrap, (8, 1))
        xe_arrs.append(xe_t)
        idx_arrs.append(idx_t)
        drel_arrs.append(drel_t.astype(BF16))
        bc = np.full(NPAD, -1.0, np.float32)
        bc[:NLOC] = batch[c * NLOC:(c + 1) * NLOC].astype(np.float32)
        sel = (bc.reshape(NPAIR, 128).T[:, :, None]
               == np.arange(G, dtype=np.float32)[None, None, :]).astype(BF16)
        sel_arrs.append(np.ascontiguousarray(sel.reshape(128, NPAIR * G)))
    return xe_arrs, idx_arrs, drel_arrs, sel_arrs


def kernel(**inputs):
    from concourse import bass_utils

    x = np.asarray(inputs["x"], dtype=np.float32)
    batch = np.asarray(inputs["batch"], dtype=np.int64)
    dinv, inv_cnt, plan, per_core = _preprocess(
        np.asarray(inputs["edge_index"]), batch)

    key = (tuple(plan["f_chunks"]), tuple(plan["a_chunks"]),
           tuple(plan["b_chunks"]))
    if key not in _CACHE:
        _CACHE.clear()
        _CACHE[key] = _build_program(plan)
    nc = _CACHE[key]

    b1r = np.asarray(inputs["b1e"], np.float32).reshape(1, C).astype(BF16)
    b2r = np.asarray(inputs["b2e"], np.float32).reshape(1, C).astype(BF16)
    ones1 = np.ones((1, C), np.float32).astype(BF16)
    blinb = np.tile(np.asarray(inputs["blin"], np.float32), (G, 1))
    identb = np.eye(C, dtype=np.float32).astype(BF16)
    iotab = np.tile(np.arange(C, dtype=np.float32), (C, 1)).astype(BF16)

    xs = x * dinv[:, None]          # D^{-1/2} X
    xe_arrs, idx_arrs, drel_arrs, sel_arrs = _host_arrays(
        plan, per_core, batch, xs)

    in_maps = []
    for c in range(NCORES):
        lo = c * NLOC
        x1l = np.zeros((C, NPAD), np.float32)
        x1l[:, :NLOC] = (x[lo:lo + NLOC]
                         * dinv[lo:lo + NLOC][:, None]).T
        dv_flat = np.zeros(NPAD, np.float32)
        dv_flat[:NLOC] = dinv[lo:lo + NLOC]
        dwp = dv_flat.reshape(NPAIR, 128).T.copy()
        in_maps.append({
            "xedge": xe_arrs[c],
            "xTl1": x1l.astype(BF16),
            "dinvrow": np.tile(dv_flat, (C, 1)).astype(BF16),
            "idx16": idx_arrs[c], "drelb": drel_arrs[c],
            "selg": sel_arrs[c],
            "identb": identb, "iotab": iotab,
            "w1e": np.asarray(inputs["W1e"], np.float32).astype(BF16),
            "w2e": np.asarray(inputs["W2e"], np.float32).astype(BF16),
            "wlin": np.asarray(inputs["Wlin"], np.float32).astype(BF16),
            "b1row": b1r, "b2row": b2r, "ones1": ones1,
            "dinvw": dwp,
            "blinb": blinb, "invcnt": inv_cnt.reshape(G, 1),
        })

    trace = bool(inputs.get("_trace", False))
    last_err = None
    for _attempt in range(3):
        try:
            res = bass_utils.run_bass_kernel_spmd(nc, in_maps,
                                                  core_ids=list(range(NCORES)),
                                                  trace=trace)
            kernel._last = res
            return np.asarray(res.results[0]["out"], dtype=np.float32)
        except Exception as e:  # transient device-state failures: retry
            last_err = e
    raise last_err


# revision 22
# speedup vs baseline: 1.1139x; 1.0155x over previous
"""Trainium2 Bass kernel for the DiffPool-style GCN forward pass.

Computation (dead softmax/pool branches of the reference are skipped):
    x1 = relu(Dh (A+I) Dh (x @ W1e) + b1e)
    x2 = relu(Dh (A+I) Dh (x1 @ W2e) + b2e)
    out = (graph_mean_pool(x2) @ Wlin) + blin          -> [64, 10] fp32

Aggregation is reassociated so raw features are aggregated first and the
dense W matmul runs once per dst window. The self-loop term is folded into
the PSUM accumulation as an identity-matmul seed (x*dinv for layer 1,
dinv*x1 for layer 2), so each window epilogue is a single DVE multiply.

Structure (v6 = v1's proven 2-phase layout + layer-1 speedups):
- Layer 1 runs on 64-wide dst windows with a host-prepared fp8
  edge-aligned stream (no gather); one-hot scatter matrices built on DVE.
- Layer 2 gathers dinv*x1 rows (bf16, SWDGE dma_gather, 1024 rows/call,
  4 queues) from tables built by two AllGathers (lo = local rows
  [0,2560), hi = rest), in TWO 128-wide window passes so the hi
  AllGather overlaps the lo pass. Partials accumulate via identity-seeded
  PSUM with ACT drains (ACT/PE never contend with the SWDGE Q7 port).
- Mean-pool partials are reduced with one AllReduce.
"""

import numpy as np
import ml_dtypes

N = 50000
E = 800000
G = 64
C = 128
C_OUT = 10
NCORES = 8
NLOC = N // NCORES          # 6250
WD1 = 64                    # layer-1 dst window width
W1 = (NLOC + WD1 - 1) // WD1   # 98
NPAIR = (W1 + 1) // 2       # 49 pairs (128 dst each)
WD2 = 128                   # layer-2 dst window width
W2 = (NLOC + WD2 - 1) // WD2   # 49
NPAD = NPAIR * 128          # 6272
LOCUT = 2176                # lo/hi split; 8*HILEN=32592 just fits int16
HILEN = NLOC - LOCUT        # 3690
NTABLO = LOCUT * NCORES
NTABHI = HILEN * NCORES
MAX_CALL_CHUNKS = 8
XE_SLAB = 32
OH_GROUP = 32
NQ = 4

BF16 = ml_dtypes.bfloat16
FP8 = ml_dtypes.float8_e4m3fn

_CACHE = {}


def _build_program(plan):
    import concourse.bacc as bacc
    import concourse.mybir as mybir
    import concourse.tile as tile
    from concourse import library_config
    from concourse.bass_interp import get_hw_module
    from concourse.tile_rust import add_dep_helper

    f32 = mybir.dt.float32
    bf16 = mybir.dt.bfloat16
    fp8 = mybir.dt.float8e4
    i16 = mybir.dt.int16
    Relu = mybir.ActivationFunctionType.Relu
    Copy = mybir.ActivationFunctionType.Copy
    Mult = mybir.AluOpType.mult

    f_chunks = plan["f_chunks"]      # [W1] layer-1 chunks per window
    win_f_base = plan["win_f_base"]
    TCH = plan["TCH"]                # aligned to OH_GROUP
    a_chunks = plan["a_chunks"]      # [W2] layer-2 lo
    b_chunks = plan["b_chunks"]      # [W2] hi
    calls_lo = plan["calls_lo"]
    calls_hi = plan["calls_hi"]
    win_lo_base = plan["win_lo_base"]
    win_hi_base = plan["win_hi_base"]
    TL = plan["TL"]
    TC = plan["total_drel_cols"]
    TIC = plan["total_idxcols"]

    nc = bacc.Bacc("TRN2", target_bir_lowering=False, debug=False,
                   num_devices=NCORES, num_swdge_queues=NQ)

    # ---- I/O ----
    xe_in = nc.dram_tensor("xedge", [C, TCH * C], fp8, kind="ExternalInput")
    xtl1_in = nc.dram_tensor("xTl1", [C, NPAD], bf16, kind="ExternalInput")
    dvr_in = nc.dram_tensor("dinvrow", [C, NPAD], bf16, kind="ExternalInput")
    idx_in = nc.dram_tensor("idx16", [C, TIC], i16, kind="ExternalInput")
    drel_in = nc.dram_tensor("drelb", [C, TC], bf16, kind="ExternalInput")
    iota_in = nc.dram_tensor("iotab", [C, C], bf16, kind="ExternalInput")
    sel_in = nc.dram_tensor("selg", [C, NPAIR * G], bf16, kind="ExternalInput")
    ident_in = nc.dram_tensor("identb", [C, C], bf16, kind="ExternalInput")
    w1_in = nc.dram_tensor("w1e", [C, C], bf16, kind="ExternalInput")
    w2_in = nc.dram_tensor("w2e", [C, C], bf16, kind="ExternalInput")
    wlin_in = nc.dram_tensor("wlin", [C, C_OUT], bf16, kind="ExternalInput")
    b1_in = nc.dram_tensor("b1row", [1, C], bf16, kind="ExternalInput")
    b2_in = nc.dram_tensor("b2row", [1, C], bf16, kind="ExternalInput")
    ones_in = nc.dram_tensor("ones1", [1, C], bf16, kind="ExternalInput")
    dinvw_in = nc.dram_tensor("dinvw", [C, NPAIR], f32, kind="ExternalInput")
    blin_in = nc.dram_tensor("blinb", [G, C_OUT], f32, kind="ExternalInput")
    icnt_in = nc.dram_tensor("invcnt", [G, 1], f32, kind="ExternalInput")
    out_t = nc.dram_tensor("out", [G, C_OUT], f32, kind="ExternalOutput")

    with tile.TileContext(nc) as tc:
        with tc.tile_pool(name="res", bufs=1) as res, \
             tc.tile_pool(name="gp", bufs=6) as gp, \
             tc.tile_pool(name="xep", bufs=3) as xep, \
             tc.tile_pool(name="ohp", bufs=3) as ohp, \
             tc.tile_pool(name="hx", bufs=4) as hxp, \
             tc.tile_pool(name="psw", bufs=2, space="PSUM") as psw, \
             tc.tile_pool(name="psd", bufs=2, space="PSUM") as psd, \
             tc.tile_pool(name="pstr", bufs=1, space="PSUM") as pstr, \
             tc.tile_pool(name="psp", bufs=1, space="PSUM") as psp, \
             tc.tile_pool(name="dram", bufs=1, space="DRAM") as dram:

            lib = nc.gpsimd.load_library(library_config.mlp)

            def load_res(name, src, shape, dt=f32):
                t = res.tile(shape, dt, tag=name, name=name)
                nc.sync.dma_start(out=t[:], in_=src[:])
                return t

            drel = load_res("r_drel", drel_in, [C, TC], bf16)
            iota = load_res("r_iota", iota_in, [C, C], bf16)
            xTl1 = load_res("r_xtl1", xtl1_in, [C, NPAD], bf16)
            dinvrow = load_res("r_dvr", dvr_in, [C, NPAD], bf16)
            identb = load_res("r_id", ident_in, [C, C], bf16)
            w1 = load_res("r_w1", w1_in, [C, C], bf16)
            w2 = load_res("r_w2", w2_in, [C, C], bf16)
            wlin = load_res("r_wl", wlin_in, [C, C_OUT], bf16)
            bias1 = load_res("r_b1", b1_in, [1, C], bf16)
            bias2 = load_res("r_b2", b2_in, [1, C], bf16)
            ones1 = load_res("r_on", ones_in, [1, C], bf16)
            dinvw = load_res("r_dw", dinvw_in, [C, NPAIR])
            blinb = load_res("r_bl", blin_in, [G, C_OUT])
            icnt = load_res("r_ic", icnt_in, [G, 1])

            x1P = res.tile([C, NPAIR * C], bf16)  # dinv*x1, [dst, f]/pair
            x1T1 = res.tile([C, NPAD], bf16)   # (dinv*x1)^T (pass-A seed)
            accT = res.tile([C, NPAD], bf16)   # pass-A partials + self

            # ---- DRAM buffers ----
            ag2a_in = dram.tile([LOCUT, C], bf16)
            ag2b_in = dram.tile([HILEN, C], bf16)
            lo2 = dram.tile([NTABLO, C], bf16, addr_space="Shared")
            hi2 = dram.tile([NTABHI, C], bf16, addr_space="Shared")
            ar_in = dram.tile([C, G], f32)
            ar_out = dram.tile([C, G], f32)
            rg = [list(range(NCORES))]

            def allgather(src, dst):
                nc.gpsimd.collective_compute(
                    "AllGather", mybir.AluOpType.bypass, replica_groups=rg,
                    ins=[src.opt()], outs=[dst.opt()])

            # ---- one-hot builder: cols < TCH are 64 wide, rest 128 ----
            ohtiles = {}

            def ensure_oh(gidx):
                g0 = (gidx // OH_GROUP) * OH_GROUP
                oht = ohtiles.get(g0)
                wd = WD1 if g0 < TCH else WD2
                if oht is None:
                    take = min(OH_GROUP, TC - g0)
                    oht = ohp.tile([C, OH_GROUP * WD2], bf16, tag="oh",
                                   name="ohbuf")
                    dcols = drel[:, g0:g0 + take]
                    nc.vector.tensor_tensor(
                        out=oht[:, 0:take * wd]
                            .rearrange("p (k m) -> p k m", m=wd),
                        in0=dcols.unsqueeze(2).to_broadcast([C, take, wd]),
                        in1=iota[:, 0:wd].unsqueeze(1)
                            .to_broadcast([C, take, wd]),
                        op=mybir.AluOpType.is_equal)
                    ohtiles[g0] = oht
                    if len(ohtiles) > 2:
                        ohtiles.pop(next(iter(ohtiles)))
                return oht, (gidx - g0), wd

            # ---- layer-1 fp8 edge stream ----
            xetiles = {}

            def ensure_xe(s):
                g0 = (s // XE_SLAB) * XE_SLAB
                xt_ = xetiles.get(g0)
                if xt_ is None:
                    take = min(XE_SLAB, TCH - g0)
                    xt_ = xep.tile([C, XE_SLAB * C], fp8, tag="xe",
                                   name="xebuf")
                    nc.sync.dma_start(out=xt_[:, 0:take * C],
                                      in_=xe_in[:, g0 * C:(g0 + take) * C])
                    xetiles[g0] = xt_
                    if len(xetiles) > 2:
                        xetiles.pop(next(iter(xetiles)))
                return xt_, s - g0

            # ===== layer 1: 64-wide windows, pairs of 2, fused dense =====
            stage2 = {"p0": 0, "ag2a": None}
            idx16 = None
            selg = None

            def flush2(pend):
                """Flush x1P pairs [p0, pend) to ag2a/ag2b (split LOCUT)."""
                p0 = stage2["p0"]
                if pend <= p0:
                    return
                r0 = p0 * 128
                r1 = min(pend * 128, NLOC)
                r = r0
                while r < r1:
                    if r < LOCUT:
                        tab, trow, end = ag2a_in, r, min(r1, LOCUT)
                    else:
                        tab, trow, end = ag2b_in, r - LOCUT, r1
                    nrow = end - r
                    k0 = r // 128
                    nfull = nrow // 128
                    if nfull > 0:
                        nc.sync.dma_start(
                            out=tab[trow:trow + nfull * 128, :]
                                .rearrange("(k p) c -> p k c", p=128),
                            in_=x1P[:, k0 * C:(k0 + nfull) * C]
                                .rearrange("p (k c) -> p k c", c=C))
                    if nfull * 128 < nrow:
                        rem = nrow - nfull * 128
                        nc.sync.dma_start(
                            out=tab[trow + nfull * 128:trow + nrow, :],
                            in_=x1P[0:rem,
                                    (k0 + nfull) * C:(k0 + nfull + 1) * C])
                    r = end
                stage2["p0"] = pend

            for pair in range(NPAIR):
                zt = hxp.tile([C, C], bf16, tag="z")
                for wi in range(2):
                    w = 2 * pair + wi
                    cw = f_chunks[w]
                    cols = slice(w * WD1, (w + 1) * WD1)
                    psA = psw.tile([C, WD2], f32, space="PSUM", tag="pw")
                    nc.tensor.matmul(out=psA[:, 0:WD1], lhsT=identb[:],
                                     rhs=xTl1[:, cols],
                                     start=True, stop=(cw == 0))
                    for k in range(cw):
                        sidx = win_f_base[w] + k
                        xe_t, xoff = ensure_xe(sidx)
                        oht, ooff, wd = ensure_oh(sidx)
                        nc.tensor.matmul(
                            out=psA[:, 0:WD1],
                            lhsT=xe_t[:, xoff * C:(xoff + 1) * C],
                            rhs=oht[:, ooff * wd:(ooff + 1) * wd],
                            start=False, stop=(k == cw - 1))
                    zc = slice(wi * WD1, (wi + 1) * WD1)
                    nc.vector.tensor_tensor(out=zt[:, zc], in0=psA[:, 0:WD1],
                                            in1=dinvrow[:, cols], op=Mult)
                ps2 = psd.tile([C, C], f32, space="PSUM", tag="pd")
                nc.tensor.matmul(out=ps2[:], lhsT=ones1[:], rhs=bias1[:],
                                 start=True, stop=False)
                nc.tensor.matmul(out=ps2[:], lhsT=zt[:], rhs=w1[:],
                                 start=False, stop=True)
                # x1 rows: dinv * x1, [dst, f] (flushed to the AG tables)
                nc.scalar.activation(x1P[:, pair * C:(pair + 1) * C],
                                     ps2[:], Relu,
                                     scale=dinvw[:, pair:pair + 1])
                pt = pstr.tile([C, C], bf16, space="PSUM", tag="tps")
                nc.tensor.transpose(out=pt[:],
                                    in_=x1P[:, pair * C:(pair + 1) * C],
                                    identity=identb[:])
                nc.scalar.activation(x1T1[:, pair * C:(pair + 1) * C],
                                     pt[:], Copy)
                if (pair + 1) % 4 == 0 or pair == NPAIR - 1 \
                        or (pair + 1) * 128 == LOCUT:
                    flush2(pair + 1)
                    if (pair + 1) * 128 >= LOCUT and stage2["ag2a"] is None:
                        allgather(ag2a_in, lo2)
                        idx16 = load_res("r_idx", idx_in, [C, TIC], i16)
                        selg = load_res("r_sel", sel_in,
                                        [C, NPAIR * G], bf16)
                        stage2["ag2a"] = True
            allgather(ag2b_in, hi2)

            # ===== layer 2: gather-based, two 128-wide passes =====
            state = {"tiles": {}, "next": [0, 0], "ci": [0]}

            def ensure_chunk(half, s):
                calls = calls_lo if half == 0 else calls_hi
                while True:
                    for (h2, st), (gt, nch) in state["tiles"].items():
                        if h2 == half and st <= s < st + nch:
                            return gt, s - st
                    st, nch, col = calls[state["next"][half]]
                    state["next"][half] += 1
                    gt = gp.tile([C, MAX_CALL_CHUNKS * C], bf16, tag="g",
                                 name="gbuf")
                    src_ap = lo2[:] if half == 0 else hi2[:]
                    ni = nch * 128
                    ci = state["ci"][0]
                    state["ci"][0] += 1
                    gi = nc.gpsimd.dma_gather(
                        gt[:, 0:nch * C].rearrange("p (k d) -> p k d", d=C),
                        src_ap, idx16[:, col:col + nch * 8],
                        ni, ni, C, single_packet=True, queue_num=ci % NQ)
                    add_dep_helper(gi.ins, lib.ins, False, "needs mlp lib")
                    state["tiles"][(half, st)] = (gt, nch)
                    if len(state["tiles"]) > 8:
                        state["tiles"].pop(next(iter(state["tiles"])))

            ps_pool = psp.tile([C, G], f32, space="PSUM", tag="pool")

            def edge_pass(half, base, cnt, colbase, final):
                for w in range(W2):
                    cw = cnt[w]
                    cols = slice(w * WD2, (w + 1) * WD2)
                    ps = psw.tile([C, WD2], f32, space="PSUM", tag="pw")
                    seed = accT if final else x1T1
                    nc.tensor.matmul(out=ps[:], lhsT=identb[:],
                                     rhs=seed[:, cols],
                                     start=True, stop=(cw == 0))
                    for k in range(cw):
                        gt, off = ensure_chunk(half, base[w] + k)
                        oht, ooff, wd = ensure_oh(colbase + base[w] + k)
                        nc.tensor.matmul(
                            out=ps[:],
                            lhsT=gt[:, off * C:(off + 1) * C],
                            rhs=oht[:, ooff * wd:(ooff + 1) * wd],
                            start=False, stop=(k == cw - 1))
                    if not final:
                        nc.scalar.activation(accT[:, cols], ps[:], Copy)
                    else:
                        zb = hxp.tile([C, C], bf16, tag="z")
                        nc.vector.tensor_tensor(out=zb[:], in0=ps[:],
                                                in1=dinvrow[:, cols],
                                                op=Mult)
                        ps2 = psd.tile([C, C], f32, space="PSUM", tag="pd")
                        nc.tensor.matmul(out=ps2[:], lhsT=ones1[:],
                                         rhs=bias2[:], start=True, stop=False)
                        nc.tensor.matmul(out=ps2[:], lhsT=zb[:], rhs=w2[:],
                                         start=False, stop=True)
                        x2t = hxp.tile([C, C], bf16, tag="xt")
                        nc.scalar.activation(x2t[:], ps2[:], Relu)
                        nc.tensor.matmul(out=ps_pool[:], lhsT=x2t[:],
                                         rhs=selg[:, w * G:(w + 1) * G],
                                         start=(w == 0), stop=(w == W2 - 1))

            edge_pass(0, win_lo_base, a_chunks, TCH, final=False)
            edge_pass(1, win_hi_base, b_chunks, TCH + TL, final=True)

            # ===== pooled all-reduce + final linear =====
            poolT = res.tile([C, G], f32)
            nc.vector.tensor_copy(out=poolT[:], in_=ps_pool[:])
            nc.sync.dma_start(out=ar_in[:], in_=poolT[:])
            nc.gpsimd.collective_compute(
                "AllReduce", mybir.AluOpType.add, replica_groups=rg,
                ins=[ar_in.opt()], outs=[ar_out.opt()])
            poolS = res.tile([C, G], f32)
            nc.sync.dma_start(out=poolS[:], in_=ar_out[:])
            poolb = res.tile([C, G], bf16)
            nc.vector.tensor_copy(out=poolb[:], in_=poolS[:])
            ps_f = psd.tile([G, C_OUT], f32, space="PSUM", tag="pd")
            nc.tensor.matmul(out=ps_f[:], lhsT=poolb[:], rhs=wlin[:],
                             start=True, stop=True)
            fin = res.tile([G, C_OUT], f32)
            nc.vector.tensor_scalar_mul(fin[:], in0=ps_f[:], scalar1=icnt[:])
            nc.vector.tensor_add(out=fin[:], in0=fin[:], in1=blinb[:])
            nc.sync.dma_start(out=out_t[:], in_=fin[:])

    nc.compile()
    nc.m = get_hw_module(nc.m)
    return nc


def _preprocess(edge_index, batch):
    src = np.asarray(edge_index[0], dtype=np.int64)
    dst = np.asarray(edge_index[1], dtype=np.int64)
    batch = np.asarray(batch, dtype=np.int64)

    deg = np.bincount(dst, minlength=N).astype(np.float64) + 1.0
    dinv = (1.0 / np.sqrt(deg)).astype(np.float32)
    counts = np.bincount(batch, minlength=G).astype(np.float64)
    inv_cnt = (1.0 / np.maximum(counts, 1.0)).astype(np.float32)

    order = np.argsort(dst, kind="stable")
    src_s = src[order]
    dst_s = dst[order]
    core_lo = np.searchsorted(dst_s, np.arange(NCORES) * NLOC)
    core_hi = np.searchsorted(dst_s, (np.arange(NCORES) + 1) * NLOC)

    per_core = []
    f_cnt = np.zeros((NCORES, W1), np.int64)
    a_cnt = np.zeros((NCORES, W2), np.int64)
    b_cnt = np.zeros((NCORES, W2), np.int64)
    for c in range(NCORES):
        s = src_s[core_lo[c]:core_hi[c]]
        d = dst_s[core_lo[c]:core_hi[c]] - c * NLOC
        owner = s // NLOC
        pos = s - owner * NLOC
        is_lo = pos < LOCUT
        row = np.where(is_lo, owner * LOCUT + pos,
                       owner * HILEN + (pos - LOCUT))
        win1 = d // WD1
        win2 = d // WD2
        w1lo = np.searchsorted(win1, np.arange(W1))
        w1hi = np.searchsorted(win1, np.arange(W1) + 1)
        w2lo = np.searchsorted(win2, np.arange(W2))
        w2hi = np.searchsorted(win2, np.arange(W2) + 1)
        wins1 = []
        for w in range(W1):
            sl = slice(w1lo[w], w1hi[w])
            wins1.append((s[sl], d[sl] - w * WD1))
            f_cnt[c, w] = w1hi[w] - w1lo[w]
        wins2 = []
        for w in range(W2):
            sl = slice(w2lo[w], w2hi[w])
            dw = d[sl] - w * WD2
            il = is_lo[sl]
            rw = row[sl]
            wins2.append((rw[il], dw[il], rw[~il], dw[~il]))
            a_cnt[c, w] = int(il.sum())
            b_cnt[c, w] = (w2hi[w] - w2lo[w]) - a_cnt[c, w]
        per_core.append((wins1, wins2))

    f_chunks = [int(-(-f_cnt[:, w].max() // 128)) for w in range(W1)]
    win_f_base = np.concatenate([[0], np.cumsum(f_chunks)])[:W1].astype(int).tolist()
    TCH = int(sum(f_chunks))
    TCH = ((TCH + OH_GROUP - 1) // OH_GROUP) * OH_GROUP  # align for builder

    a_chunks = [int(-(-a_cnt[:, w].max() // 128)) for w in range(W2)]
    b_chunks = [int(-(-b_cnt[:, w].max() // 128)) for w in range(W2)]
    win_lo_base = np.concatenate([[0], np.cumsum(a_chunks)])[:W2].astype(int).tolist()
    win_hi_base = np.concatenate([[0], np.cumsum(b_chunks)])[:W2].astype(int).tolist()
    TL = int(sum(a_chunks))
    TH = int(sum(b_chunks))
    total_drel_cols = TCH + TL + TH

    calls_lo, calls_hi = [], []
    idx_col = 0
    for done_target, calls in ((TL, calls_lo), (TH, calls_hi)):
        done = 0
        while done < done_target:
            take = min(MAX_CALL_CHUNKS, done_target - done)
            calls.append((done, take, idx_col))
            idx_col += take * 8
            done += take
    total_idxcols = idx_col

    plan = {"f_chunks": f_chunks, "win_f_base": win_f_base, "TCH": TCH,
            "a_chunks": a_chunks, "b_chunks": b_chunks,
            "calls_lo": calls_lo, "calls_hi": calls_hi,
            "win_lo_base": win_lo_base, "win_hi_base": win_hi_base,
            "TL": TL, "total_drel_cols": total_drel_cols,
            "total_idxcols": total_idxcols}

    return dinv, inv_cnt, plan, per_core


def _host_arrays(plan, per_core, batch, xs):
    f_chunks = plan["f_chunks"]
    a_chunks = plan["a_chunks"]
    b_chunks = plan["b_chunks"]
    win_f_base = plan["win_f_base"]
    win_lo_base = plan["win_lo_base"]
    win_hi_base = plan["win_hi_base"]
    TCH = plan["TCH"]
    TL = plan["TL"]
    TC = plan["total_drel_cols"]
    TIC = plan["total_idxcols"]
    TH = TC - TCH - TL

    xe_arrs, idx_arrs, drel_arrs, sel_arrs = [], [], [], []
    xsb = xs.astype(FP8)
    for c in range(NCORES):
        wins1, wins2 = per_core[c]
        xe_t = np.zeros((128, TCH * C), FP8)
        drel_t = np.full((128, TC), -1.0, np.float32)
        lo_idx = np.zeros(TL * 128, np.int16)
        hi_idx = np.zeros(TH * 128, np.int16)
        for w in range(W1):
            sw, dw = wins1[w]
            o = win_f_base[w]
            nr = len(sw)
            nch = f_chunks[w]
            buf = np.zeros((nch * 128, C), FP8)
            buf[:nr] = xsb[sw]
            xe_t[:, o * C:(o + nch) * C] = \
                buf.reshape(nch, 128, C).transpose(1, 0, 2).reshape(128, nch * C)
            fl = np.full(nch * 128, -1.0, np.float32)
            fl[:nr] = dw.astype(np.float32)
            drel_t[:, o:o + nch] = fl.reshape(nch, 128).T
        for w in range(W2):
            rw_lo, dw_lo, rw_hi, dw_hi = wins2[w]
            o = win_lo_base[w]
            lo_idx[o * 128:o * 128 + len(rw_lo)] = rw_lo.astype(np.int16)
            fl = np.full(a_chunks[w] * 128, -1.0, np.float32)
            fl[:len(dw_lo)] = dw_lo.astype(np.float32)
            drel_t[:, TCH + o:TCH + o + a_chunks[w]] = \
                fl.reshape(a_chunks[w], 128).T
            o = win_hi_base[w]
            hi_idx[o * 128:o * 128 + len(rw_hi)] = rw_hi.astype(np.int16)
            fh = np.full(b_chunks[w] * 128, -1.0, np.float32)
            fh[:len(dw_hi)] = dw_hi.astype(np.float32)
            drel_t[:, TCH + TL + o:TCH + TL + o + b_chunks[w]] = \
                fh.reshape(b_chunks[w], 128).T
        idx_t = np.zeros((128, TIC), np.int16)
        for half, calls in ((0, plan["calls_lo"]), (1, plan["calls_hi"])):
            src_idx = lo_idx if half == 0 else hi_idx
            for s0, take, col in calls:
                seg = src_idx[s0 * 128:(s0 + take) * 128]
                wrap = seg.reshape(take * 8, 16).T
                idx_t[:, col:col + take * 8] = np.tile(wrap, (8, 1))
        xe_arrs.append(xe_t)
        idx_arrs.append(idx_t)
        drel_arrs.append(drel_t.astype(BF16))
        bc = np.full(NPAD, -1.0, np.float32)
        bc[:NLOC] = batch[c * NLOC:(c + 1) * NLOC].astype(np.float32)
        sel = (bc.reshape(NPAIR, 128).T[:, :, None]
               == np.arange(G, dtype=np.float32)[None, None, :]).astype(BF16)
        sel_arrs.append(np.ascontiguousarray(sel.reshape(128, NPAIR * G)))
    return xe_arrs, idx_arrs, drel_arrs, sel_arrs


def kernel(**inputs):
    from concourse import bass_utils

    x = np.asarray(inputs["x"], dtype=np.float32)
    batch = np.asarray(inputs["batch"], dtype=np.int64)
    dinv, inv_cnt, plan, per_core = _preprocess(
        np.asarray(inputs["edge_index"]), batch)

    key = (tuple(plan["f_chunks"]), tuple(plan["a_chunks"]),
           tuple(plan["b_chunks"]))
    if key not in _CACHE:
        _CACHE.clear()
        _CACHE[key] = _build_program(plan)
    nc = _CACHE[key]

    b1r = np.asarray(inputs["b1e"], np.float32).reshape(1, C).astype(BF16)
    b2r = np.asarray(inputs["b2e"], np.float32).reshape(1, C).astype(BF16)
    ones1 = np.ones((1, C), np.float32).astype(BF16)
    blinb = np.tile(np.asarray(inputs["blin"], np.float32), (G, 1))
    identb = np.eye(C, dtype=np.float32).astype(BF16)
    iotab = np.tile(np.arange(C, dtype=np.float32), (C, 1)).astype(BF16)

    xs = x * dinv[:, None]          # D^{-1/2} X
    xe_arrs, idx_arrs, drel_arrs, sel_arrs = _host_arrays(
        plan, per_core, batch, xs)

    in_maps = []
    for c in range(NCORES):
        lo = c * NLOC
        x1l = np.zeros((C, NPAD), np.float32)
        x1l[:, :NLOC] = (x[lo:lo + NLOC]
                         * dinv[lo:lo + NLOC][:, None]).T
        dv_flat = np.zeros(NPAD, np.float32)
        dv_flat[:NLOC] = dinv[lo:lo + NLOC]
        dwp = dv_flat.reshape(NPAIR, 128).T.copy()
        in_maps.append({
            "xedge": xe_arrs[c],
            "xTl1": x1l.astype(BF16),
            "dinvrow": np.tile(dv_flat, (C, 1)).astype(BF16),
            "idx16": idx_arrs[c], "drelb": drel_arrs[c],
            "selg": sel_arrs[c],
            "identb": identb, "iotab": iotab,
            "w1e": np.asarray(inputs["W1e"], np.float32).astype(BF16),
            "w2e": np.asarray(inputs["W2e"], np.float32).astype(BF16),
            "wlin": np.asarray(inputs["Wlin"], np.float32).astype(BF16),
            "b1row": b1r, "b2row": b2r, "ones1": ones1,
            "dinvw": dwp,
            "blinb": blinb, "invcnt": inv_cnt.reshape(G, 1),
        })

    trace = bool(inputs.get("_trace", False))
    last_err = None
    for _attempt in range(3):
        try:
            res = bass_utils.run_bass_kernel_spmd(nc, in_maps,
                                                  core_ids=list(range(NCORES)),
                                                  trace=trace)
            kernel._last = res
            return np.asarray(res.results[0]["out"], dtype=np.float32)
        except Exception as e:  # transient device-state failures: retry
            last_err = e
    raise last_err
